# revision 11
# baseline (speedup 1.0000x reference)
"""ALBEF concept-text contrastive loss on 8 TRN2 NeuronCores (v4).

Design (per core r, owning batch rows r*32:(r+1)*32):
  * S computed ONCE per core as fp8 DoubleRow matmuls (K=256 in one pass):
    pair-tiles [128=(qo,i), 1024] hold two m-chunks (cols 0:480 / 512:992),
    values scaled by 256 (features quantized to fp8e4 with x16 scale each).
  * term_col (sum_q max_l S): DVE segmented reduce_max from PSUM into
    colmax[m] [128, 256] bf16; 8 eones matmuls -> term_col [32, 256] PSUM.
  * term_row (masked mean_l max_q S): max_q as a sharp log-sum-exp with
    beta=96: ACT exp per pair-tile -> SBUF bf16; the 4 pair-tiles are
    summed with gpsimd accumulate-DMAs; 2 matmuls with one-hot Eq fold
    sum_q over partitions -> qs [32, w] PSUM; qs copied into qsf. Ln,
    mask-weighting and the segmented l-sum are batched per column third
    (one Exp<->Ln ACT table switch pair per third).
  * Text features projected locally, fp8-converted, AllGathered in 3
    pipelined 480/480/320 column groups (Shared outputs) into a per-rank
    contiguous layout.
  * Loss: local rows give row-lse and diag; column exp-sums and the local
    scalar combine with a single 257-wide AllReduce; all cores compute
    the same final scalar.
"""

import ml_dtypes
import numpy as np

import concourse.bass as bass
import concourse.bacc as bacc
import concourse.mybir as mybir
import concourse.tile as tile
from concourse.bass_utils import run_bass_kernel_spmd

F32 = mybir.dt.float32
BF16 = mybir.dt.bfloat16
FP8 = mybir.dt.float8e4
AX = mybir.AxisListType
ALU = mybir.AluOpType
ACTF = mybir.ActivationFunctionType
PM = mybir.MatmulPerfMode

B, Q, L, VW, TW, D = 256, 32, 40, 768, 768, 256
NCORES = 8
BL = B // NCORES            # 32 local batch rows
IQ = BL * Q                 # 1024 local (q,i) columns, q-major
JLL = BL * L                # 1280 local (j,l) columns
JL = B * L                  # 10240 global (j,l)
KC = VW // 128              # 6 contraction chunks for projection
NM = IQ // 128              # 8 m-chunks of concept rows

FSC = 16.0                  # fp8 feature scale (S scaled by FSC^2=256)
BHAT = 0.375                # exp scale on scaled S (beta_orig = 96)
SHIFT = 12.0                # exp arg = BHAT*S_tilde - SHIFT

# main-pass chunks within each rank's 1280 columns: (offset, width, nj)
CCHUNKS = [(0, 480, 12), (480, 480, 12), (960, 320, 8)]
# AllGather groups: (offset, width, text-proj m-chunk that completes it)
AGROUPS = [(0, 480, 3), (480, 480, 7), (960, 320, 9)]

_CACHE = {}


def _build():
    nc = _build_graph()
    nc.compile()
    return nc


def _build_graph():
    import os
    nc = bacc.Bacc("TRN2", target_bir_lowering=False, debug=False,
                   num_devices=NCORES)

    concept_t = nc.dram_tensor("concept_t", [VW, IQ], BF16, kind="ExternalInput")
    text_t = nc.dram_tensor("text_t", [TW, JLL], BF16, kind="ExternalInput")
    wc = nc.dram_tensor("wc", [VW, D], BF16, kind="ExternalInput")
    ww = nc.dram_tensor("ww", [TW, D], BF16, kind="ExternalInput")
    brows = nc.dram_tensor("brows", [1, 2 * D], BF16, kind="ExternalInput")
    ones_row = nc.dram_tensor("ones_row", [1, 128], BF16, kind="ExternalInput")
    ident_bf = nc.dram_tensor("ident_bf", [128, 128], BF16, kind="ExternalInput")
    eqmat = nc.dram_tensor("eqmat", [128, BL], BF16, kind="ExternalInput")
    eones = nc.dram_tensor("eones", [128, BL], BF16, kind="ExternalInput")
    maskw = nc.dram_tensor("maskw", [BL, JL], BF16, kind="ExternalInput")
    dmask = nc.dram_tensor("dmask", [BL, B], F32, kind="ExternalInput")
    ones32 = nc.dram_tensor("ones32", [BL, 1], F32, kind="ExternalInput")

    out = nc.dram_tensor("out", [1, 1], F32, kind="ExternalOutput")
    dbg = None
    if os.environ.get("KDBG"):
        dbg = nc.dram_tensor("dbg", [BL, B], F32, kind="ExternalOutput")

    # collective buffers (outputs Shared for fast HBM-HBM paths)
    ag_in = [nc.dram_tensor(f"ag_in{g}", [2 * 128, w], FP8, kind="Internal")
             for g, (_, w, _) in enumerate(AGROUPS)]
    ag_out = [nc.dram_tensor(f"ag_out{g}", [NCORES * 2 * 128, w], FP8,
                             kind="Internal", addr_space="Shared")
              for g, (_, w, _) in enumerate(AGROUPS)]
    ar_in = nc.dram_tensor("ar_in", [1, B + 1], F32, kind="Internal")
    ar_out = nc.dram_tensor("ar_out", [1, B + 1], F32, kind="Internal",
                            addr_space="Shared")

    with tile.TileContext(nc) as tc:
        with (
            tc.tile_pool(name="cst", bufs=1) as cst,
            tc.tile_pool(name="feat", bufs=1) as feat,
        ):
            # ---- persistent SBUF tiles ----
            cf8 = feat.tile([128, 2, IQ], FP8, tag="cf8")
            wfl8 = feat.tile([128, 2, JLL], FP8, tag="wfl8")
            wf8g = feat.tile([128, 2, JL], FP8, tag="wf8g")
            colmax = [feat.tile([128, B], BF16, tag=f"colmax{m}",
                                name=f"colmax{m}") for m in range(NM)]
            trow_sb = feat.tile([BL, B], F32, tag="trow_sb")
            sim_sb = feat.tile([BL, B], F32, tag="sim_sb")
            qsf = feat.tile([BL, JL], F32, tag="qsf")

            ident_sb = cst.tile([128, 128], BF16, tag="ident_sb")
            eq_sb = cst.tile([128, BL], BF16, tag="eq_sb")
            eones_sb = cst.tile([128, BL], BF16, tag="eones_sb")
            maskw_sb = cst.tile([BL, JL], BF16, tag="maskw_sb")
            dmask_sb = cst.tile([BL, B], F32, tag="dmask_sb")
            ones32_sb = cst.tile([BL, 1], F32, tag="ones32_sb")
            ones32b_sb = cst.tile([BL, 1], BF16, tag="ones32b_sb")
            onesr_sb = cst.tile([1, 128], BF16, tag="onesr_sb")
            brow_sb = cst.tile([1, 2 * D], BF16, tag="brow_sb")
            shift_sb = cst.tile([128, 1], F32, tag="shift_sb")

            nc.scalar.dma_start(ident_sb[:], ident_bf[:])
            nc.scalar.dma_start(eq_sb[:], eqmat[:])
            nc.scalar.dma_start(eones_sb[:], eones[:])
            nc.scalar.dma_start(dmask_sb[:], dmask[:])
            nc.scalar.dma_start(ones32_sb[:], ones32[:])
            nc.vector.memset(ones32b_sb[:], 1.0)
            nc.vector.memset(shift_sb[:], -SHIFT)
            nc.scalar.dma_start(onesr_sb[:], ones_row[:])
            nc.scalar.dma_start(brow_sb[:], brows[:])
            nc.scalar.dma_start(maskw_sb[:], maskw[:])
            # preload Square/Sqrt/Exp/Ln tables while DMAs run
            warm = cst.tile([1, 4], F32, tag="warm")
            nc.scalar.activation(warm[0:1, 3:4], ones32_sb[0:1, :], ACTF.Square)
            nc.scalar.activation(warm[0:1, 0:1], ones32_sb[0:1, :], ACTF.Sqrt)
            nc.scalar.activation(warm[0:1, 1:2], ones32_sb[0:1, :], ACTF.Exp)
            nc.scalar.activation(warm[0:1, 2:3], ones32_sb[0:1, :], ACTF.Ln)

            def issue_ag(g):
                off, w, _ = AGROUPS[g]
                for k in range(2):
                    nc.sync.dma_start(ag_in[g][k * 128:(k + 1) * 128, :],
                                      wfl8[:, k, off:off + w])
                nc.gpsimd.collective_compute(
                    "AllGather", ALU.bypass,
                    ins=[ag_in[g][:]], outs=[ag_out[g][:]],
                    replica_groups=[list(range(NCORES))])
                for rr in range(NCORES):
                    for k in range(2):
                        nc.sync.dma_start(
                            wf8g[:, k, rr * JLL + off:rr * JLL + off + w],
                            ag_out[g][rr * 256 + k * 128:
                                      rr * 256 + (k + 1) * 128, :])

            # ---- stage 1: projections + l2norm -> fp8 transposed feats ----
            with (
                tc.tile_pool(name="pin", bufs=1) as pin,
                tc.tile_pool(name="ps2", bufs=3, space="PSUM") as ps2,
                tc.tile_pool(name="pst", bufs=4, space="PSUM") as pst,
                tc.tile_pool(name="wk2", bufs=3) as wk2,
            ):
                tin = pin.tile([128, KC * JLL], BF16, tag="tin")
                cin = pin.tile([128, KC * IQ], BF16, tag="cin")
                wcs = pin.tile([128, KC * D], BF16, tag="wcs")
                wws = pin.tile([128, KC * D], BF16, tag="wws")
                for k in range(KC):
                    nc.sync.dma_start(tin[:, k * JLL:(k + 1) * JLL],
                                      text_t[k * 128:(k + 1) * 128, :])
                    nc.sync.dma_start(wws[:, k * D:(k + 1) * D],
                                      ww[k * 128:(k + 1) * 128, :])
                for k in range(KC):
                    nc.sync.dma_start(cin[:, k * IQ:(k + 1) * IQ],
                                      concept_t[k * 128:(k + 1) * 128, :])
                    nc.sync.dma_start(wcs[:, k * D:(k + 1) * D],
                                      wc[k * 128:(k + 1) * 128, :])

                def project(src, width, w_sb, brow_ix, dst8, after_m=None):
                    for m in range(width // 128):
                        pp = ps2.tile([128, D], F32, tag="pp")
                        for k in range(KC):
                            nc.tensor.matmul(
                                pp[:],
                                lhsT=src[:, k * width + m * 128:
                                         k * width + (m + 1) * 128],
                                rhs=w_sb[:, k * D:(k + 1) * D],
                                start=(k == 0), stop=False)
                        nc.tensor.matmul(
                            pp[:], lhsT=onesr_sb[:],
                            rhs=brow_sb[0:1, brow_ix * D:(brow_ix + 1) * D],
                            start=False, stop=True)
                        sq = wk2.tile([128, D], BF16, tag="sq")
                        ss = wk2.tile([128, 1], F32, tag="ss")
                        nc.scalar.activation(sq[:], pp[:], ACTF.Square,
                                             accum_out=ss[:])
                        rcp = wk2.tile([128, 1], F32, tag="rcp")
                        nc.vector.reciprocal(rcp[:], ss[:])
                        rn = wk2.tile([128, 1], F32, tag="rn")
                        # rn = FSC / sqrt(ss)
                        nc.scalar.activation(rn[:], rcp[:], ACTF.Sqrt,
                                             scale=FSC * FSC)
                        nrm = wk2.tile([128, D], BF16, tag="nrm")
                        nc.scalar.activation(nrm[:], pp[:], ACTF.Copy,
                                             scale=rn[:])
                        for kk in range(2):
                            ptr = pst.tile([128, 128], BF16, tag="ptr")
                            nc.tensor.transpose(
                                ptr[:], nrm[:, kk * 128:(kk + 1) * 128],
                                ident_sb[:])
                            nc.scalar.copy(
                                dst8[:, kk, m * 128:(m + 1) * 128], ptr[:])
                        if after_m is not None and m in after_m:
                            after_m[m]()

                project(tin, JLL, wws, 1, wfl8,
                        after_m={mm: (lambda g=g: issue_ag(g))
                                 for g, (_, _, mm) in enumerate(AGROUPS)})
                project(cin, IQ, wcs, 0, cf8)

            # ---- stage 2: main pass ----
            with (
                tc.tile_pool(name="ptc", bufs=1, space="PSUM") as ptc,
                tc.tile_pool(name="psa", bufs=2, space="PSUM") as psa,
                tc.tile_pool(name="psq", bufs=1, space="PSUM") as psq,
                tc.tile_pool(name="wke", bufs=6) as wke,
                tc.tile_pool(name="wkl", bufs=3) as wkl,
            ):
                term_col = ptc.tile([BL, B], F32, tag="term_col")

                def do_chunk(ci, rr):
                    coff, w, nj = CCHUNKS[ci]
                    goff = rr * JLL + coff
                    jg0 = rr * BL + (coff // L)
                    exs = []
                    for pr in range(4):
                        pa = psa.tile([128, 1024], F32, tag="pa")
                        for h in range(2):
                            m = 2 * pr + h
                            nc.tensor.matmul(
                                pa[:, h * 512:h * 512 + w],
                                lhsT=cf8[:, :, m * 128:(m + 1) * 128],
                                rhs=wf8g[:, :, goff:goff + w],
                                start=True, stop=True,
                                perf_mode=PM.DoubleRow)
                            nc.vector.tensor_reduce(
                                colmax[m][:, jg0:jg0 + nj],
                                pa[:, h * 512:h * 512 + w].rearrange(
                                    "p (j l) -> p j l", l=L),
                                axis=AX.X, op=ALU.max)
                        ex = wke.tile([128, 1024], BF16, tag="ex")
                        nc.scalar.activation(ex[:], pa[:], ACTF.Exp,
                                             scale=BHAT, bias=shift_sb[:])
                        exs.append(ex)
                    # presum the 4 pair-tiles on the DMA engines (gpsimd DGE)
                    for k in range(1, 4):
                        nc.gpsimd.dma_start(exs[0][:], exs[k][:],
                                            accum_op=ALU.add)
                    qs = psq.tile([BL, 512], F32, tag="qs")
                    for h in range(2):
                        nc.tensor.matmul(qs[0:BL, 0:w],
                                         lhsT=eq_sb[:],
                                         rhs=exs[0][:, h * 512:h * 512 + w],
                                         start=(h == 0), stop=(h == 1))
                    nc.scalar.copy(qsf[:, goff:goff + w], qs[0:BL, 0:w])

                def ln_batch(ci):
                    coff, w, nj = CCHUNKS[ci]
                    for rr in range(NCORES):
                        goff = rr * JLL + coff
                        jg0 = rr * BL + (coff // L)
                        lns = wkl.tile([BL, w], BF16, tag="lns")
                        nc.scalar.activation(lns[:], qsf[:, goff:goff + w],
                                             ACTF.Ln)
                        lnw = wkl.tile([BL, w], BF16, tag="lnw")
                        nc.vector.tensor_tensor(
                            lnw[:], lns[:], maskw_sb[:, goff:goff + w],
                            op=ALU.mult)
                        nc.vector.tensor_reduce(
                            trow_sb[:, jg0:jg0 + nj],
                            lnw[:].rearrange("p (j l) -> p j l", l=L),
                            axis=AX.X, op=ALU.add)

                for ci in range(len(CCHUNKS)):
                    for rr in range(NCORES):
                        do_chunk(ci, rr)
                    ln_batch(ci)

                # ---- stage 3: term_col + sim ----
                for m in range(NM):
                    nc.tensor.matmul(term_col[:], lhsT=eones_sb[:],
                                     rhs=colmax[m][:],
                                     start=(m == 0), stop=(m == NM - 1))
                nc.vector.tensor_tensor(sim_sb[:], term_col[:], trow_sb[:],
                                        op=ALU.add)
                if dbg is not None:
                    nc.sync.dma_start(dbg[:], sim_sb[:])

            # ---- loss ----
            with (
                tc.tile_pool(name="ps7", bufs=1, space="PSUM") as ps7,
                tc.tile_pool(name="wk7", bufs=1) as wk7,
            ):
                nrmax = wk7.tile([BL, 1], F32, tag="nrmax")
                nc.vector.tensor_reduce(nrmax[:], sim_sb[:], axis=AX.X,
                                        op=ALU.max, negate=True)
                escr = wk7.tile([BL, B], F32, tag="escr")
                sume = wk7.tile([BL, 1], F32, tag="sume")
                nc.scalar.activation(escr[:], sim_sb[:], ACTF.Exp,
                                     bias=nrmax[:], scale=1.0,
                                     accum_out=sume[:])
                lg = wk7.tile([BL, 1], F32, tag="lg")
                nc.scalar.activation(lg[:], sume[:], ACTF.Ln)
                dscr = wk7.tile([BL, B], F32, tag="dscr")
                dg = wk7.tile([BL, 1], F32, tag="dg")
                nc.vector.scalar_tensor_tensor(
                    dscr[:], sim_sb[:], 1.0, dmask_sb[:],
                    op0=ALU.mult, op1=ALU.mult, accum_out=dg[:])
                # v = 2*dg - (lg - nrmax) = 2*dg - lg + nrmax  [BL,1]
                v1 = wk7.tile([BL, 1], F32, tag="v1")
                nc.vector.scalar_tensor_tensor(
                    v1[:], dg[:], 2.0, lg[:],
                    op0=ALU.mult, op1=ALU.subtract)
                v2 = wk7.tile([BL, 1], F32, tag="v2")
                nc.vector.tensor_tensor(v2[:], v1[:], nrmax[:], op=ALU.add)
                # column partials: sum_i e^{sim[i, j]} (no shift)
                ecol = wk7.tile([BL, B], BF16, tag="ecol")
                nc.scalar.activation(ecol[:], sim_sb[:], ACTF.Exp)
                csum = ps7.tile([1, B], F32, tag="csum")
                nc.tensor.matmul(csum[:], lhsT=ones32b_sb[:],
                                 rhs=ecol[:], start=True, stop=True)
                ssum = ps7.tile([1, 1], F32, tag="ssum")
                nc.tensor.matmul(ssum[:], lhsT=ones32_sb[:], rhs=v2[:],
                                 start=True, stop=True)
                arv = wk7.tile([1, B + 1], F32, tag="arv")
                nc.scalar.copy(arv[0:1, 0:B], csum[0:1, :])
                nc.scalar.copy(arv[0:1, B:B + 1], ssum[0:1, :])
                nc.sync.dma_start(ar_in[:], arv[:])
                nc.gpsimd.collective_compute(
                    "AllReduce", ALU.add,
                    ins=[ar_in[:]], outs=[ar_out[:]],
                    replica_groups=[list(range(NCORES))])
                arr = wk7.tile([1, B + 1], F32, tag="arr")
                nc.sync.dma_start(arr[:], ar_out[:])
                lnc = wk7.tile([1, B], F32, tag="lnc")
                lnsum = wk7.tile([1, 1], F32, tag="lnsum")
                nc.scalar.activation(lnc[:], arr[0:1, 0:B], ACTF.Ln,
                                     accum_out=lnsum[:])
                fin = wk7.tile([1, 1], F32, tag="fin")
                nc.vector.tensor_tensor(fin[:], lnsum[:],
                                        arr[0:1, B:B + 1],
                                        op=ALU.subtract)
                osb = wk7.tile([1, 1], F32, tag="osb")
                nc.scalar.mul(osb[:], fin[:], 1.0 / (2 * B))
                nc.sync.dma_start(out[:], osb[0:1, :])

    return nc


def _host_prep(inputs):
    concept_feat = np.ascontiguousarray(np.asarray(inputs["concept_feat"],
                                                   dtype=np.float32))
    text_embeds = np.array(np.asarray(inputs["text_embeds"],
                                      dtype=np.float32), copy=True)
    text_mask = np.asarray(inputs["text_mask"]).astype(np.int32)
    Wc = np.ascontiguousarray(np.asarray(inputs["Wc"], dtype=np.float32))
    bc = np.asarray(inputs["bc"], dtype=np.float32)
    Ww = np.ascontiguousarray(np.asarray(inputs["Ww"], dtype=np.float32))
    bw = np.asarray(inputs["bw"], dtype=np.float32)
    temp = float(np.asarray(inputs["temp_cpt"]))

    # word mask (drop CLS + SEP), valid counts
    m = text_mask.copy()
    m[:, 0] = 0
    sep = (L - 1) - np.argmax(m[:, ::-1] > 0, axis=1)
    m[np.arange(B), sep] = 0
    nw = m.sum(axis=1).astype(np.float32)

    # sanitize invalid text rows with copies of position 1 (always valid)
    for j in range(B):
        inv = m[j] == 0
        text_embeds[j, inv] = text_embeds[j, 1]

    # Eq: partition p=(qo,i) -> i one-hot; eones adds 1/(Q*temp*FSC^2)
    eq = np.zeros((128, BL), dtype=np.float32)
    eones = np.zeros((128, BL), dtype=np.float32)
    for p in range(128):
        eq[p, p % BL] = 1.0
        eones[p, p % BL] = 1.0 / (Q * temp * FSC * FSC)

    # maskw[i, (j,l)] = m[j,l] / (nw_j * temp * FSC^2 * BHAT), bcast over i
    w_jl = (m.astype(np.float32) /
            (nw[:, None] * temp * FSC * FSC * BHAT)).reshape(1, JL)
    maskw = np.repeat(w_jl, BL, axis=0)

    ident = np.eye(128, dtype=np.float32)
    ones_row = np.ones((1, 128), dtype=np.float32)
    ones32 = np.ones((BL, 1), dtype=np.float32)
    brows = np.concatenate([bc, bw])[None, :]

    shared = {
        "wc": Wc.astype(ml_dtypes.bfloat16), "ww": Ww.astype(ml_dtypes.bfloat16),
        "brows": brows.astype(ml_dtypes.bfloat16),
        "ones_row": ones_row.astype(ml_dtypes.bfloat16),
        "ident_bf": ident.astype(ml_dtypes.bfloat16),
        "eqmat": eq.astype(ml_dtypes.bfloat16),
        "eones": eones.astype(ml_dtypes.bfloat16),
        "maskw": maskw.astype(ml_dtypes.bfloat16),
        "ones32": ones32,
    }
    in_maps = []
    for r in range(NCORES):
        im = dict(shared)
        im["concept_t"] = np.ascontiguousarray(
            concept_feat[r * BL:(r + 1) * BL].transpose(1, 0, 2)
            .reshape(IQ, VW).T).astype(ml_dtypes.bfloat16)
        im["text_t"] = np.ascontiguousarray(
            text_embeds[r * BL:(r + 1) * BL].reshape(JLL, TW).T
        ).astype(ml_dtypes.bfloat16)
        dmask_np = np.zeros((BL, B), dtype=np.float32)
        dmask_np[np.arange(BL), r * BL + np.arange(BL)] = 1.0
        im["dmask"] = dmask_np
        in_maps.append(im)
    return in_maps


def kernel(**inputs):
    in_maps = _host_prep(inputs)
    if "nc" not in _CACHE:
        _CACHE["nc"] = _build()
    res = run_bass_kernel_spmd(_CACHE["nc"], in_maps,
                               core_ids=list(range(NCORES)))
    return np.float32(res.results[0]["out"][0, 0])


# revision 15
# speedup vs baseline: 1.0291x; 1.0291x over previous
"""ALBEF concept-text contrastive loss on 8 TRN2 NeuronCores (v4).

Design (per core r, owning batch rows r*32:(r+1)*32):
  * S computed ONCE per core as fp8 DoubleRow matmuls (K=256 in one pass):
    pair-tiles [128=(qo,i), 1024] hold two m-chunks (cols 0:480 / 512:992),
    values scaled by 256 (features quantized to fp8e4 with x16 scale each).
  * term_col (sum_q max_l S): DVE segmented reduce_max from PSUM into
    colmax[m] [128, 256] bf16; 8 eones matmuls -> term_col [32, 256] PSUM.
  * term_row (masked mean_l max_q S): max_q as a sharp log-sum-exp with
    beta=96: ACT exp per pair-tile -> SBUF bf16; the 4 pair-tiles are
    summed with gpsimd accumulate-DMAs; 2 matmuls with one-hot Eq fold
    sum_q over partitions -> qs [32, w] PSUM; qs copied into qsf. Ln,
    mask-weighting and the segmented l-sum are batched per column third
    (one Exp<->Ln ACT table switch pair per third).
  * Text features projected locally, fp8-converted, AllGathered in 3
    pipelined 480/480/320 column groups (Shared outputs) into a per-rank
    contiguous layout.
  * Loss: local rows give row-lse and diag; column exp-sums and the local
    scalar combine with a single 257-wide AllReduce; all cores compute
    the same final scalar.
"""

import ml_dtypes
import numpy as np

import concourse.bass as bass
import concourse.bacc as bacc
import concourse.mybir as mybir
import concourse.tile as tile
from concourse.bass_utils import run_bass_kernel_spmd

F32 = mybir.dt.float32
BF16 = mybir.dt.bfloat16
FP8 = mybir.dt.float8e4
AX = mybir.AxisListType
ALU = mybir.AluOpType
ACTF = mybir.ActivationFunctionType
PM = mybir.MatmulPerfMode

B, Q, L, VW, TW, D = 256, 32, 40, 768, 768, 256
NCORES = 8
BL = B // NCORES            # 32 local batch rows
IQ = BL * Q                 # 1024 local (q,i) columns, q-major
JLL = BL * L                # 1280 local (j,l) columns
JL = B * L                  # 10240 global (j,l)
KC = VW // 128              # 6 contraction chunks for projection
NM = IQ // 128              # 8 m-chunks of concept rows

FSC = 16.0                  # fp8 feature scale (S scaled by FSC^2=256)
BHAT = 0.375                # exp scale on scaled S (beta_orig = 96)
SHIFT = 12.0                # exp arg = BHAT*S_tilde - SHIFT

# main-pass chunks within each rank's 1280 columns: (offset, width, nj)
CCHUNKS = [(0, 480, 12), (480, 480, 12), (960, 320, 8)]
# AllGather groups: (offset, width, text-proj m-chunk that completes it)
AGROUPS = [(0, 480, 3), (480, 480, 7), (960, 320, 9)]

_CACHE = {}


def _build():
    nc = _build_graph()
    nc.compile()
    return nc


def _build_graph():
    import os
    nc = bacc.Bacc("TRN2", target_bir_lowering=False, debug=False,
                   num_devices=NCORES)

    concept_t = nc.dram_tensor("concept_t", [VW, IQ], BF16, kind="ExternalInput")
    text_t = nc.dram_tensor("text_t", [TW, JLL], BF16, kind="ExternalInput")
    wc = nc.dram_tensor("wc", [VW, D], BF16, kind="ExternalInput")
    ww = nc.dram_tensor("ww", [TW, D], BF16, kind="ExternalInput")
    brows = nc.dram_tensor("brows", [1, 2 * D], BF16, kind="ExternalInput")
    ones_row = nc.dram_tensor("ones_row", [1, 128], BF16, kind="ExternalInput")
    ident_bf = nc.dram_tensor("ident_bf", [128, 128], BF16, kind="ExternalInput")
    eqmat = nc.dram_tensor("eqmat", [128, BL], BF16, kind="ExternalInput")
    eones = nc.dram_tensor("eones", [128, BL], BF16, kind="ExternalInput")
    maskw = nc.dram_tensor("maskw", [BL, JL], BF16, kind="ExternalInput")
    dmask = nc.dram_tensor("dmask", [BL, B], F32, kind="ExternalInput")
    ones32 = nc.dram_tensor("ones32", [BL, 1], F32, kind="ExternalInput")

    out = nc.dram_tensor("out", [1, 1], F32, kind="ExternalOutput")
    dbg = None
    if os.environ.get("KDBG"):
        dbg = nc.dram_tensor("dbg", [BL, B], F32, kind="ExternalOutput")

    # collective buffers (outputs Shared for fast HBM-HBM paths)
    ag_in = [nc.dram_tensor(f"ag_in{g}", [2 * 128, w], FP8, kind="Internal")
             for g, (_, w, _) in enumerate(AGROUPS)]
    ag_out = [nc.dram_tensor(f"ag_out{g}", [NCORES * 2 * 128, w], FP8,
                             kind="Internal", addr_space="Shared")
              for g, (_, w, _) in enumerate(AGROUPS)]
    ar_in = nc.dram_tensor("ar_in", [1, B + 1], F32, kind="Internal")
    ar_out = nc.dram_tensor("ar_out", [1, B + 1], F32, kind="Internal",
                            addr_space="Shared")

    with tile.TileContext(nc) as tc:
        with (
            tc.tile_pool(name="cst", bufs=1) as cst,
            tc.tile_pool(name="feat", bufs=1) as feat,
        ):
            # ---- persistent SBUF tiles ----
            cf8 = feat.tile([128, 2, IQ], FP8, tag="cf8")
            wfl8 = feat.tile([128, 2, JLL], FP8, tag="wfl8")
            wf8g = feat.tile([128, 2, JL], FP8, tag="wf8g")
            colmax = [feat.tile([128, B], BF16, tag=f"colmax{m}",
                                name=f"colmax{m}") for m in range(NM)]
            trow_sb = feat.tile([BL, B], F32, tag="trow_sb")
            sim_sb = feat.tile([BL, B], F32, tag="sim_sb")
            qsf = feat.tile([BL, JL], F32, tag="qsf")

            ident_sb = cst.tile([128, 128], BF16, tag="ident_sb")
            eq_sb = cst.tile([128, BL], BF16, tag="eq_sb")
            eones_sb = cst.tile([128, BL], BF16, tag="eones_sb")
            maskw_sb = cst.tile([BL, JL], BF16, tag="maskw_sb")
            dmask_sb = cst.tile([BL, B], F32, tag="dmask_sb")
            ones32_sb = cst.tile([BL, 1], F32, tag="ones32_sb")
            ones32b_sb = cst.tile([BL, 1], BF16, tag="ones32b_sb")
            onesr_sb = cst.tile([1, 128], BF16, tag="onesr_sb")
            brow_sb = cst.tile([1, 2 * D], BF16, tag="brow_sb")
            shift_sb = cst.tile([128, 1], F32, tag="shift_sb")

            nc.scalar.dma_start(ident_sb[:], ident_bf[:])
            nc.scalar.dma_start(eq_sb[:], eqmat[:])
            nc.scalar.dma_start(eones_sb[:], eones[:])
            nc.scalar.dma_start(dmask_sb[:], dmask[:])
            nc.scalar.dma_start(ones32_sb[:], ones32[:])
            nc.vector.memset(ones32b_sb[:], 1.0)
            nc.vector.memset(shift_sb[:], -SHIFT)
            nc.scalar.dma_start(onesr_sb[:], ones_row[:])
            nc.scalar.dma_start(brow_sb[:], brows[:])
            nc.scalar.dma_start(maskw_sb[:], maskw[:])
            # preload Square/Sqrt/Exp/Ln tables while DMAs run
            warm = cst.tile([1, 4], F32, tag="warm")
            nc.scalar.activation(warm[0:1, 3:4], ones32_sb[0:1, :], ACTF.Square)
            nc.scalar.activation(warm[0:1, 0:1], ones32_sb[0:1, :], ACTF.Sqrt)
            nc.scalar.activation(warm[0:1, 1:2], ones32_sb[0:1, :], ACTF.Exp)
            nc.scalar.activation(warm[0:1, 2:3], ones32_sb[0:1, :], ACTF.Ln)

            def issue_ag(g):
                off, w, _ = AGROUPS[g]
                nc.sync.dma_start(
                    ag_in[g][:].rearrange("(k p) w -> p k w", p=128),
                    wfl8[:, :, off:off + w])
                nc.gpsimd.collective_compute(
                    "AllGather", ALU.bypass,
                    ins=[ag_in[g][:]], outs=[ag_out[g][:]],
                    replica_groups=[list(range(NCORES))])
                for k in range(2):
                    nc.sync.dma_start(
                        wf8g[:, k, :].rearrange("p (rr j) -> p rr j",
                                                j=JLL)[:, :, off:off + w],
                        ag_out[g][:].rearrange("(rr k p) w -> p k rr w",
                                               k=2, p=128)[:, k, :, :])

            # ---- stage 1: projections + l2norm -> fp8 transposed feats ----
            with (
                tc.tile_pool(name="pin", bufs=1) as pin,
                tc.tile_pool(name="ps2", bufs=3, space="PSUM") as ps2,
                tc.tile_pool(name="pst", bufs=4, space="PSUM") as pst,
                tc.tile_pool(name="wk2", bufs=3) as wk2,
            ):
                tin = pin.tile([128, KC * JLL], BF16, tag="tin")
                cin = pin.tile([128, KC * IQ], BF16, tag="cin")
                wcs = pin.tile([128, KC * D], BF16, tag="wcs")
                wws = pin.tile([128, KC * D], BF16, tag="wws")
                nc.sync.dma_start(
                    tin[:].rearrange("p (k j) -> p k j", j=JLL),
                    text_t[:].rearrange("(k p) j -> p k j", p=128))
                nc.sync.dma_start(
                    wws[:].rearrange("p (k d) -> p k d", d=D),
                    ww[:].rearrange("(k p) d -> p k d", p=128))
                nc.sync.dma_start(
                    cin[:].rearrange("p (k j) -> p k j", j=IQ),
                    concept_t[:].rearrange("(k p) j -> p k j", p=128))
                nc.sync.dma_start(
                    wcs[:].rearrange("p (k d) -> p k d", d=D),
                    wc[:].rearrange("(k p) d -> p k d", p=128))

                def project(src, width, w_sb, brow_ix, dst8, after_m=None):
                    for m in range(width // 128):
                        pp = ps2.tile([128, D], F32, tag="pp")
                        for k in range(KC):
                            nc.tensor.matmul(
                                pp[:],
                                lhsT=src[:, k * width + m * 128:
                                         k * width + (m + 1) * 128],
                                rhs=w_sb[:, k * D:(k + 1) * D],
                                start=(k == 0), stop=False)
                        nc.tensor.matmul(
                            pp[:], lhsT=onesr_sb[:],
                            rhs=brow_sb[0:1, brow_ix * D:(brow_ix + 1) * D],
                            start=False, stop=True)
                        sq = wk2.tile([128, D], BF16, tag="sq")
                        ss = wk2.tile([128, 1], F32, tag="ss")
                        nc.scalar.activation(sq[:], pp[:], ACTF.Square,
                                             accum_out=ss[:])
                        rcp = wk2.tile([128, 1], F32, tag="rcp")
                        nc.vector.reciprocal(rcp[:], ss[:])
                        rn = wk2.tile([128, 1], F32, tag="rn")
                        # rn = FSC / sqrt(ss)
                        nc.scalar.activation(rn[:], rcp[:], ACTF.Sqrt,
                                             scale=FSC * FSC)
                        nrm = wk2.tile([128, D], BF16, tag="nrm")
                        nc.scalar.activation(nrm[:], pp[:], ACTF.Copy,
                                             scale=rn[:])
                        for kk in range(2):
                            ptr = pst.tile([128, 128], BF16, tag="ptr")
                            nc.tensor.transpose(
                                ptr[:], nrm[:, kk * 128:(kk + 1) * 128],
                                ident_sb[:])
                            nc.scalar.copy(
                                dst8[:, kk, m * 128:(m + 1) * 128], ptr[:])
                        if after_m is not None and m in after_m:
                            after_m[m]()

                project(tin, JLL, wws, 1, wfl8,
                        after_m={mm: (lambda g=g: issue_ag(g))
                                 for g, (_, _, mm) in enumerate(AGROUPS)})
                project(cin, IQ, wcs, 0, cf8)

            # ---- stage 2: main pass ----
            with (
                tc.tile_pool(name="ptc", bufs=1, space="PSUM") as ptc,
                tc.tile_pool(name="psa", bufs=2, space="PSUM") as psa,
                tc.tile_pool(name="psq", bufs=1, space="PSUM") as psq,
                tc.tile_pool(name="wke", bufs=6) as wke,
                tc.tile_pool(name="wkl", bufs=3) as wkl,
            ):
                term_col = ptc.tile([BL, B], F32, tag="term_col")

                def do_chunk(ci, rr):
                    coff, w, nj = CCHUNKS[ci]
                    goff = rr * JLL + coff
                    jg0 = rr * BL + (coff // L)
                    exs = []
                    for pr in range(4):
                        pa = psa.tile([128, 1024], F32, tag="pa")
                        for h in range(2):
                            m = 2 * pr + h
                            nc.tensor.matmul(
                                pa[:, h * 512:h * 512 + w],
                                lhsT=cf8[:, :, m * 128:(m + 1) * 128],
                                rhs=wf8g[:, :, goff:goff + w],
                                start=True, stop=True,
                                perf_mode=PM.DoubleRow)
                            nc.vector.tensor_reduce(
                                colmax[m][:, jg0:jg0 + nj],
                                pa[:, h * 512:h * 512 + w].rearrange(
                                    "p (j l) -> p j l", l=L),
                                axis=AX.X, op=ALU.max)
                        ex = wke.tile([128, 1024], BF16, tag="ex")
                        nc.scalar.activation(ex[:], pa[:], ACTF.Exp,
                                             scale=BHAT, bias=shift_sb[:])
                        exs.append(ex)
                    # presum the 4 pair-tiles on the DMA engines (gpsimd DGE)
                    for k in range(1, 4):
                        nc.gpsimd.dma_start(exs[0][:], exs[k][:],
                                            accum_op=ALU.add)
                    qs = psq.tile([BL, 512], F32, tag="qs")
                    for h in range(2):
                        nc.tensor.matmul(qs[0:BL, 0:w],
                                         lhsT=eq_sb[:],
                                         rhs=exs[0][:, h * 512:h * 512 + w],
                                         start=(h == 0), stop=(h == 1))
                    nc.vector.tensor_scalar_mul(qsf[:, goff:goff + w],
                                                qs[0:BL, 0:w], 1.0)

                def ln_batch(ci, tok):
                    coff, w, nj = CCHUNKS[ci]
                    for rr in range(NCORES):
                        goff = rr * JLL + coff
                        jg0 = rr * BL + (coff // L)
                        lns = wkl.tile([BL, w], BF16, tag="lns")
                        # scale=tok (==1.0) only delays Ln past the last
                        # chunk of this ci so ACT Exp<->Ln table reloads
                        # happen once per ci instead of per chunk
                        nc.scalar.activation(lns[:], qsf[:, goff:goff + w],
                                             ACTF.Ln, scale=tok[:])
                        lnw = wkl.tile([BL, w], BF16, tag="lnw")
                        nc.vector.tensor_tensor(
                            lnw[:], lns[:], maskw_sb[:, goff:goff + w],
                            op=ALU.mult)
                        nc.vector.tensor_reduce(
                            trow_sb[:, jg0:jg0 + nj],
                            lnw[:].rearrange("p (j l) -> p j l", l=L),
                            axis=AX.X, op=ALU.add)

                for ci in range(len(CCHUNKS)):
                    for rr in range(NCORES):
                        do_chunk(ci, rr)
                    coff, w, _ = CCHUNKS[ci]
                    lastc = (NCORES - 1) * JLL + coff
                    tok = wkl.tile([BL, 1], F32, tag="tok")
                    nc.vector.scalar_tensor_tensor(
                        tok[:], qsf[:, lastc:lastc + 1], 0.0, ones32_sb[:],
                        op0=ALU.mult, op1=ALU.add)
                    ln_batch(ci, tok)

                # ---- stage 3: term_col + sim ----
                for m in range(NM):
                    nc.tensor.matmul(term_col[:], lhsT=eones_sb[:],
                                     rhs=colmax[m][:],
                                     start=(m == 0), stop=(m == NM - 1))
                nc.vector.tensor_tensor(sim_sb[:], term_col[:], trow_sb[:],
                                        op=ALU.add)
                if dbg is not None:
                    nc.sync.dma_start(dbg[:], sim_sb[:])

            # ---- loss ----
            with (
                tc.tile_pool(name="ps7", bufs=1, space="PSUM") as ps7,
                tc.tile_pool(name="wk7", bufs=1) as wk7,
            ):
                nrmax = wk7.tile([BL, 1], F32, tag="nrmax")
                nc.vector.tensor_reduce(nrmax[:], sim_sb[:], axis=AX.X,
                                        op=ALU.max, negate=True)
                escr = wk7.tile([BL, B], F32, tag="escr")
                sume = wk7.tile([BL, 1], F32, tag="sume")
                nc.scalar.activation(escr[:], sim_sb[:], ACTF.Exp,
                                     bias=nrmax[:], scale=1.0,
                                     accum_out=sume[:])
                lg = wk7.tile([BL, 1], F32, tag="lg")
                nc.scalar.activation(lg[:], sume[:], ACTF.Ln)
                dscr = wk7.tile([BL, B], F32, tag="dscr")
                dg = wk7.tile([BL, 1], F32, tag="dg")
                nc.vector.scalar_tensor_tensor(
                    dscr[:], sim_sb[:], 1.0, dmask_sb[:],
                    op0=ALU.mult, op1=ALU.mult, accum_out=dg[:])
                # v = 2*dg - (lg - nrmax) = 2*dg - lg + nrmax  [BL,1]
                v1 = wk7.tile([BL, 1], F32, tag="v1")
                nc.vector.scalar_tensor_tensor(
                    v1[:], dg[:], 2.0, lg[:],
                    op0=ALU.mult, op1=ALU.subtract)
                v2 = wk7.tile([BL, 1], F32, tag="v2")
                nc.vector.tensor_tensor(v2[:], v1[:], nrmax[:], op=ALU.add)
                # column partials: sum_i e^{sim[i, j]} (no shift)
                ecol = wk7.tile([BL, B], BF16, tag="ecol")
                nc.scalar.activation(ecol[:], sim_sb[:], ACTF.Exp)
                csum = ps7.tile([1, B], F32, tag="csum")
                nc.tensor.matmul(csum[:], lhsT=ones32b_sb[:],
                                 rhs=ecol[:], start=True, stop=True)
                ssum = ps7.tile([1, 1], F32, tag="ssum")
                nc.tensor.matmul(ssum[:], lhsT=ones32_sb[:], rhs=v2[:],
                                 start=True, stop=True)
                arv = wk7.tile([1, B + 1], F32, tag="arv")
                nc.scalar.copy(arv[0:1, 0:B], csum[0:1, :])
                nc.scalar.copy(arv[0:1, B:B + 1], ssum[0:1, :])
                nc.sync.dma_start(ar_in[:], arv[:])
                nc.gpsimd.collective_compute(
                    "AllReduce", ALU.add,
                    ins=[ar_in[:]], outs=[ar_out[:]],
                    replica_groups=[list(range(NCORES))])
                arr = wk7.tile([1, B + 1], F32, tag="arr")
                nc.sync.dma_start(arr[:], ar_out[:])
                lnc = wk7.tile([1, B], F32, tag="lnc")
                lnsum = wk7.tile([1, 1], F32, tag="lnsum")
                nc.scalar.activation(lnc[:], arr[0:1, 0:B], ACTF.Ln,
                                     accum_out=lnsum[:])
                fin = wk7.tile([1, 1], F32, tag="fin")
                nc.vector.tensor_tensor(fin[:], lnsum[:],
                                        arr[0:1, B:B + 1],
                                        op=ALU.subtract)
                osb = wk7.tile([1, 1], F32, tag="osb")
                nc.scalar.mul(osb[:], fin[:], 1.0 / (2 * B))
                nc.sync.dma_start(out[:], osb[0:1, :])

    return nc


def _host_prep(inputs):
    concept_feat = np.ascontiguousarray(np.asarray(inputs["concept_feat"],
                                                   dtype=np.float32))
    text_embeds = np.array(np.asarray(inputs["text_embeds"],
                                      dtype=np.float32), copy=True)
    text_mask = np.asarray(inputs["text_mask"]).astype(np.int32)
    Wc = np.ascontiguousarray(np.asarray(inputs["Wc"], dtype=np.float32))
    bc = np.asarray(inputs["bc"], dtype=np.float32)
    Ww = np.ascontiguousarray(np.asarray(inputs["Ww"], dtype=np.float32))
    bw = np.asarray(inputs["bw"], dtype=np.float32)
    temp = float(np.asarray(inputs["temp_cpt"]))

    # word mask (drop CLS + SEP), valid counts
    m = text_mask.copy()
    m[:, 0] = 0
    sep = (L - 1) - np.argmax(m[:, ::-1] > 0, axis=1)
    m[np.arange(B), sep] = 0
    nw = m.sum(axis=1).astype(np.float32)

    # sanitize invalid text rows with copies of position 1 (always valid)
    for j in range(B):
        inv = m[j] == 0
        text_embeds[j, inv] = text_embeds[j, 1]

    # Eq: partition p=(qo,i) -> i one-hot; eones adds 1/(Q*temp*FSC^2)
    eq = np.zeros((128, BL), dtype=np.float32)
    eones = np.zeros((128, BL), dtype=np.float32)
    for p in range(128):
        eq[p, p % BL] = 1.0
        eones[p, p % BL] = 1.0 / (Q * temp * FSC * FSC)

    # maskw[i, (j,l)] = m[j,l] / (nw_j * temp * FSC^2 * BHAT), bcast over i
    w_jl = (m.astype(np.float32) /
            (nw[:, None] * temp * FSC * FSC * BHAT)).reshape(1, JL)
    maskw = np.repeat(w_jl, BL, axis=0)

    ident = np.eye(128, dtype=np.float32)
    ones_row = np.ones((1, 128), dtype=np.float32)
    ones32 = np.ones((BL, 1), dtype=np.float32)
    brows = np.concatenate([bc, bw])[None, :]

    shared = {
        "wc": Wc.astype(ml_dtypes.bfloat16), "ww": Ww.astype(ml_dtypes.bfloat16),
        "brows": brows.astype(ml_dtypes.bfloat16),
        "ones_row": ones_row.astype(ml_dtypes.bfloat16),
        "ident_bf": ident.astype(ml_dtypes.bfloat16),
        "eqmat": eq.astype(ml_dtypes.bfloat16),
        "eones": eones.astype(ml_dtypes.bfloat16),
        "maskw": maskw.astype(ml_dtypes.bfloat16),
        "ones32": ones32,
    }
    in_maps = []
    for r in range(NCORES):
        im = dict(shared)
        im["concept_t"] = np.ascontiguousarray(
            concept_feat[r * BL:(r + 1) * BL].transpose(1, 0, 2)
            .reshape(IQ, VW).T).astype(ml_dtypes.bfloat16)
        im["text_t"] = np.ascontiguousarray(
            text_embeds[r * BL:(r + 1) * BL].reshape(JLL, TW).T
        ).astype(ml_dtypes.bfloat16)
        dmask_np = np.zeros((BL, B), dtype=np.float32)
        dmask_np[np.arange(BL), r * BL + np.arange(BL)] = 1.0
        im["dmask"] = dmask_np
        in_maps.append(im)
    return in_maps


def kernel(**inputs):
    in_maps = _host_prep(inputs)
    if "nc" not in _CACHE:
        _CACHE["nc"] = _build()
    res = run_bass_kernel_spmd(_CACHE["nc"], in_maps,
                               core_ids=list(range(NCORES)))
    return np.float32(res.results[0]["out"][0, 0])


# revision 16
# speedup vs baseline: 1.2143x; 1.1799x over previous
"""ALBEF concept-text contrastive loss on 8 TRN2 NeuronCores (v4).

Design (per core r, owning batch rows r*32:(r+1)*32):
  * S computed ONCE per core as fp8 DoubleRow matmuls (K=256 in one pass):
    pair-tiles [128=(qo,i), 1024] hold two m-chunks (cols 0:480 / 512:992),
    values scaled by 256 (features quantized to fp8e4 with x16 scale each).
  * term_col (sum_q max_l S): DVE segmented reduce_max from PSUM into
    colmax[m] [128, 256] bf16; 8 eones matmuls -> term_col [32, 256] PSUM.
  * term_row (masked mean_l max_q S): max_q as a sharp log-sum-exp with
    beta=96: ACT exp per pair-tile -> SBUF bf16; the 4 pair-tiles are
    summed with gpsimd accumulate-DMAs; 2 matmuls with one-hot Eq fold
    sum_q over partitions -> qs [32, w] PSUM; qs copied into qsf. Ln,
    mask-weighting and the segmented l-sum are batched per column third
    (one Exp<->Ln ACT table switch pair per third).
  * Text features projected locally, fp8-converted, AllGathered in 3
    pipelined 480/480/320 column groups (Shared outputs) into a per-rank
    contiguous layout.
  * Loss: local rows give row-lse and diag; column exp-sums and the local
    scalar combine with a single 257-wide AllReduce; all cores compute
    the same final scalar.
"""

import ml_dtypes
import numpy as np

import concourse.bass as bass
import concourse.bacc as bacc
import concourse.mybir as mybir
import concourse.tile as tile
from concourse.bass_utils import run_bass_kernel_spmd

F32 = mybir.dt.float32
BF16 = mybir.dt.bfloat16
FP8 = mybir.dt.float8e4
AX = mybir.AxisListType
ALU = mybir.AluOpType
ACTF = mybir.ActivationFunctionType
PM = mybir.MatmulPerfMode

B, Q, L, VW, TW, D = 256, 32, 40, 768, 768, 256
NCORES = 8
BL = B // NCORES            # 32 local batch rows
IQ = BL * Q                 # 1024 local (q,i) columns, q-major
JLL = BL * L                # 1280 local (j,l) columns
JL = B * L                  # 10240 global (j,l)
KC = VW // 128              # 6 contraction chunks for projection
NM = IQ // 128              # 8 m-chunks of concept rows

FSC = 16.0                  # fp8 feature scale (S scaled by FSC^2=256)
BHAT = 0.375                # exp scale on scaled S (beta_orig = 96)
SHIFT = 12.0                # exp arg = BHAT*S_tilde - SHIFT

# main-pass chunks within each rank's 1280 columns: (offset, width, nj)
CCHUNKS = [(0, 480, 12), (480, 480, 12), (960, 320, 8)]
# AllGather groups: (offset, width, text-proj m-chunk that completes it)
AGROUPS = [(0, 480, 3), (480, 480, 7), (960, 320, 9)]

_CACHE = {}


def _build():
    nc = _build_graph()
    nc.compile()
    return nc


def _build_graph():
    import os
    nc = bacc.Bacc("TRN2", target_bir_lowering=False, debug=False,
                   num_devices=NCORES)

    concept_t = nc.dram_tensor("concept_t", [VW, IQ], BF16, kind="ExternalInput")
    text_t = nc.dram_tensor("text_t", [TW, JLL], BF16, kind="ExternalInput")
    wc = nc.dram_tensor("wc", [VW, D], BF16, kind="ExternalInput")
    ww = nc.dram_tensor("ww", [TW, D], BF16, kind="ExternalInput")
    brows = nc.dram_tensor("brows", [1, 2 * D], BF16, kind="ExternalInput")
    ones_row = nc.dram_tensor("ones_row", [1, 128], BF16, kind="ExternalInput")
    ident_bf = nc.dram_tensor("ident_bf", [128, 128], BF16, kind="ExternalInput")
    eqmat = nc.dram_tensor("eqmat", [128, BL], BF16, kind="ExternalInput")
    eones = nc.dram_tensor("eones", [128, BL], BF16, kind="ExternalInput")
    maskw = nc.dram_tensor("maskw", [BL, JL], BF16, kind="ExternalInput")
    dmask = nc.dram_tensor("dmask", [BL, B], F32, kind="ExternalInput")
    ones32 = nc.dram_tensor("ones32", [BL, 1], F32, kind="ExternalInput")

    out = nc.dram_tensor("out", [1, 1], F32, kind="ExternalOutput")
    dbg = None
    if os.environ.get("KDBG"):
        dbg = nc.dram_tensor("dbg", [BL, B], F32, kind="ExternalOutput")

    # collective buffers (outputs Shared for fast HBM-HBM paths)
    ag_in = [nc.dram_tensor(f"ag_in{g}", [2 * 128, w], FP8, kind="Internal")
             for g, (_, w, _) in enumerate(AGROUPS)]
    ag_out = [nc.dram_tensor(f"ag_out{g}", [NCORES * 2 * 128, w], FP8,
                             kind="Internal", addr_space="Shared")
              for g, (_, w, _) in enumerate(AGROUPS)]
    ar_in = nc.dram_tensor("ar_in", [1, B + 1], F32, kind="Internal")
    ar_out = nc.dram_tensor("ar_out", [1, B + 1], F32, kind="Internal",
                            addr_space="Shared")

    with tile.TileContext(nc) as tc:
        with (
            tc.tile_pool(name="cst", bufs=1) as cst,
            tc.tile_pool(name="feat", bufs=1) as feat,
        ):
            # ---- persistent SBUF tiles ----
            cf8 = feat.tile([128, 2, IQ], FP8, tag="cf8")
            wfl8 = feat.tile([128, 2, JLL], FP8, tag="wfl8")
            wf8g = feat.tile([128, 2, JL], FP8, tag="wf8g")
            cm_all = feat.tile([128, NM * B], BF16, tag="cm_all")
            trow_sb = feat.tile([BL, B], F32, tag="trow_sb")
            sim_sb = feat.tile([BL, B], F32, tag="sim_sb")
            qsf = feat.tile([BL, JL], F32, tag="qsf")

            ident_sb = cst.tile([128, 128], BF16, tag="ident_sb")
            eq_sb = cst.tile([128, BL], BF16, tag="eq_sb")
            eones_sb = cst.tile([128, BL], BF16, tag="eones_sb")
            maskw_sb = cst.tile([BL, JL], BF16, tag="maskw_sb")
            dmask_sb = cst.tile([BL, B], F32, tag="dmask_sb")
            ones32_sb = cst.tile([BL, 1], F32, tag="ones32_sb")
            ones32b_sb = cst.tile([BL, 1], BF16, tag="ones32b_sb")
            onesr_sb = cst.tile([1, 128], BF16, tag="onesr_sb")
            brow_sb = cst.tile([1, 2 * D], BF16, tag="brow_sb")
            shift_sb = cst.tile([128, 1], F32, tag="shift_sb")

            nc.scalar.dma_start(ident_sb[:], ident_bf[:])
            nc.scalar.dma_start(eq_sb[:], eqmat[:])
            nc.scalar.dma_start(eones_sb[:], eones[:])
            nc.scalar.dma_start(dmask_sb[:], dmask[:])
            nc.scalar.dma_start(ones32_sb[:], ones32[:])
            nc.vector.memset(ones32b_sb[:], 1.0)
            nc.vector.memset(shift_sb[:], -SHIFT)
            nc.scalar.dma_start(onesr_sb[:], ones_row[:])
            nc.scalar.dma_start(brow_sb[:], brows[:])
            nc.scalar.dma_start(maskw_sb[:], maskw[:])
            # preload Square/Sqrt/Exp/Ln tables while DMAs run
            warm = cst.tile([1, 4], F32, tag="warm")
            nc.scalar.activation(warm[0:1, 3:4], ones32_sb[0:1, :], ACTF.Square)
            nc.scalar.activation(warm[0:1, 0:1], ones32_sb[0:1, :], ACTF.Sqrt)
            nc.scalar.activation(warm[0:1, 1:2], ones32_sb[0:1, :], ACTF.Exp)
            nc.scalar.activation(warm[0:1, 2:3], ones32_sb[0:1, :], ACTF.Ln)

            def issue_ag(g):
                off, w, _ = AGROUPS[g]
                nc.sync.dma_start(
                    ag_in[g][:].rearrange("(k p) w -> p k w", p=128),
                    wfl8[:, :, off:off + w])
                nc.gpsimd.collective_compute(
                    "AllGather", ALU.bypass,
                    ins=[ag_in[g][:]], outs=[ag_out[g][:]],
                    replica_groups=[list(range(NCORES))])
                for k in range(2):
                    nc.sync.dma_start(
                        wf8g[:, k, :].rearrange("p (rr j) -> p rr j",
                                                j=JLL)[:, :, off:off + w],
                        ag_out[g][:].rearrange("(rr k p) w -> p k rr w",
                                               k=2, p=128)[:, k, :, :])

            # ---- stage 1: projections + l2norm -> fp8 transposed feats ----
            with (
                tc.tile_pool(name="pin", bufs=1) as pin,
                tc.tile_pool(name="ps2", bufs=3, space="PSUM") as ps2,
                tc.tile_pool(name="pst", bufs=4, space="PSUM") as pst,
                tc.tile_pool(name="wk2", bufs=3) as wk2,
            ):
                tin = pin.tile([128, KC * JLL], BF16, tag="tin")
                cin = pin.tile([128, KC * IQ], BF16, tag="cin")
                wcs = pin.tile([128, KC * D], BF16, tag="wcs")
                wws = pin.tile([128, KC * D], BF16, tag="wws")
                nc.sync.dma_start(
                    tin[:].rearrange("p (k j) -> p k j", j=JLL),
                    text_t[:].rearrange("(k p) j -> p k j", p=128))
                nc.sync.dma_start(
                    wws[:].rearrange("p (k d) -> p k d", d=D),
                    ww[:].rearrange("(k p) d -> p k d", p=128))
                nc.sync.dma_start(
                    cin[:].rearrange("p (k j) -> p k j", j=IQ),
                    concept_t[:].rearrange("(k p) j -> p k j", p=128))
                nc.sync.dma_start(
                    wcs[:].rearrange("p (k d) -> p k d", d=D),
                    wc[:].rearrange("(k p) d -> p k d", p=128))

                def project(src, width, w_sb, brow_ix, dst8, ms,
                            after_m=None):
                    for m in ms:
                        pp = ps2.tile([128, D], F32, tag="pp")
                        for k in range(KC):
                            nc.tensor.matmul(
                                pp[:],
                                lhsT=src[:, k * width + m * 128:
                                         k * width + (m + 1) * 128],
                                rhs=w_sb[:, k * D:(k + 1) * D],
                                start=(k == 0), stop=False)
                        nc.tensor.matmul(
                            pp[:], lhsT=onesr_sb[:],
                            rhs=brow_sb[0:1, brow_ix * D:(brow_ix + 1) * D],
                            start=False, stop=True)
                        sq = wk2.tile([128, D], BF16, tag="sq")
                        ss = wk2.tile([128, 1], F32, tag="ss")
                        nc.scalar.activation(sq[:], pp[:], ACTF.Square,
                                             accum_out=ss[:])
                        rcp = wk2.tile([128, 1], F32, tag="rcp")
                        nc.vector.reciprocal(rcp[:], ss[:])
                        rn = wk2.tile([128, 1], F32, tag="rn")
                        # rn = FSC / sqrt(ss)
                        nc.scalar.activation(rn[:], rcp[:], ACTF.Sqrt,
                                             scale=FSC * FSC)
                        nrm = wk2.tile([128, D], BF16, tag="nrm")
                        nc.scalar.activation(nrm[:], pp[:], ACTF.Copy,
                                             scale=rn[:])
                        for kk in range(2):
                            ptr = pst.tile([128, 128], BF16, tag="ptr")
                            nc.tensor.transpose(
                                ptr[:], nrm[:, kk * 128:(kk + 1) * 128],
                                ident_sb[:])
                            nc.scalar.copy(
                                dst8[:, kk, m * 128:(m + 1) * 128], ptr[:])
                        if after_m is not None and m in after_m:
                            after_m[m]()

                project(tin, JLL, wws, 1, wfl8, ms=range(0, 4),
                        after_m={3: lambda: issue_ag(0)})
                project(cin, IQ, wcs, 0, cf8, ms=range(NM))
                project(tin, JLL, wws, 1, wfl8, ms=range(4, 10),
                        after_m={7: lambda: issue_ag(1),
                                 9: lambda: issue_ag(2)})

            # ---- stage 2: main pass ----
            with (
                tc.tile_pool(name="ptc", bufs=1, space="PSUM") as ptc,
                tc.tile_pool(name="psa", bufs=3, space="PSUM") as psa,
                tc.tile_pool(name="psq", bufs=1, space="PSUM") as psq,
                tc.tile_pool(name="wke", bufs=10) as wke,
                tc.tile_pool(name="wkl", bufs=3) as wkl,
            ):
                term_col = ptc.tile([BL, B], F32, tag="term_col")

                def do_chunk(ci, rr):
                    coff, w, nj = CCHUNKS[ci]
                    goff = rr * JLL + coff
                    jg0 = rr * BL + (coff // L)
                    exs = []
                    for pr in range(4):
                        pa = psa.tile([128, 1024], F32, tag="pa")
                        for h in range(2):
                            m = 2 * pr + h
                            nc.tensor.matmul(
                                pa[:, h * 512:h * 512 + w],
                                lhsT=cf8[:, :, m * 128:(m + 1) * 128],
                                rhs=wf8g[:, :, goff:goff + w],
                                start=True, stop=True,
                                perf_mode=PM.DoubleRow)
                        # one segmented max over l for both m-halves
                        nc.vector.tensor_reduce(
                            cm_all[:].rearrange("p (m j) -> p m j", j=B)[
                                :, 2 * pr:2 * pr + 2, jg0:jg0 + nj],
                            pa[:].rearrange("p (h x) -> p h x", x=512)[
                                :, :, 0:w].rearrange(
                                "p h (j l) -> p h j l", l=L),
                            axis=AX.X, op=ALU.max)
                        ex = wke.tile([128, 1024], BF16, tag="ex")
                        nc.scalar.activation(ex[:], pa[:], ACTF.Exp,
                                             scale=BHAT, bias=shift_sb[:])
                        exs.append(ex)
                    # presum the 4 pair-tiles on the DMA engines (gpsimd DGE)
                    for k in range(1, 4):
                        nc.gpsimd.dma_start(exs[0][:], exs[k][:],
                                            accum_op=ALU.add)
                    qs = psq.tile([BL, 512], F32, tag="qs")
                    for h in range(2):
                        nc.tensor.matmul(qs[0:BL, 0:w],
                                         lhsT=eq_sb[:],
                                         rhs=exs[0][:, h * 512:h * 512 + w],
                                         start=(h == 0), stop=(h == 1))
                    nc.vector.tensor_scalar_mul(qsf[:, goff:goff + w],
                                                qs[0:BL, 0:w], 1.0)

                def ln_batch(ci, tok):
                    coff, w, nj = CCHUNKS[ci]
                    for rr in range(NCORES):
                        goff = rr * JLL + coff
                        jg0 = rr * BL + (coff // L)
                        lns = wkl.tile([BL, w], BF16, tag="lns")
                        # scale=tok (==1.0) only delays Ln past the last
                        # chunk of this ci so ACT Exp<->Ln table reloads
                        # happen once per ci instead of per chunk
                        nc.scalar.activation(lns[:], qsf[:, goff:goff + w],
                                             ACTF.Ln, scale=tok[:])
                        lnw = wkl.tile([BL, w], BF16, tag="lnw")
                        nc.vector.tensor_tensor(
                            lnw[:], lns[:], maskw_sb[:, goff:goff + w],
                            op=ALU.mult)
                        nc.vector.tensor_reduce(
                            trow_sb[:, jg0:jg0 + nj],
                            lnw[:].rearrange("p (j l) -> p j l", l=L),
                            axis=AX.X, op=ALU.add)

                for ci in range(len(CCHUNKS)):
                    for rr in range(NCORES):
                        do_chunk(ci, rr)
                    coff, w, _ = CCHUNKS[ci]
                    lastc = (NCORES - 1) * JLL + coff
                    tok = wkl.tile([BL, 1], F32, tag="tok")
                    nc.vector.scalar_tensor_tensor(
                        tok[:], qsf[:, lastc:lastc + 1], 0.0, ones32_sb[:],
                        op0=ALU.mult, op1=ALU.add)
                    ln_batch(ci, tok)

                # ---- stage 3: term_col + sim ----
                for m in range(NM):
                    nc.tensor.matmul(term_col[:], lhsT=eones_sb[:],
                                     rhs=cm_all[:, m * B:(m + 1) * B],
                                     start=(m == 0), stop=(m == NM - 1))
                nc.vector.tensor_tensor(sim_sb[:], term_col[:], trow_sb[:],
                                        op=ALU.add)
                if dbg is not None:
                    nc.sync.dma_start(dbg[:], sim_sb[:])

            # ---- loss ----
            with (
                tc.tile_pool(name="ps7", bufs=1, space="PSUM") as ps7,
                tc.tile_pool(name="wk7", bufs=1) as wk7,
            ):
                nrmax = wk7.tile([BL, 1], F32, tag="nrmax")
                nc.vector.tensor_reduce(nrmax[:], sim_sb[:], axis=AX.X,
                                        op=ALU.max, negate=True)
                escr = wk7.tile([BL, B], F32, tag="escr")
                sume = wk7.tile([BL, 1], F32, tag="sume")
                nc.scalar.activation(escr[:], sim_sb[:], ACTF.Exp,
                                     bias=nrmax[:], scale=1.0,
                                     accum_out=sume[:])
                lg = wk7.tile([BL, 1], F32, tag="lg")
                nc.scalar.activation(lg[:], sume[:], ACTF.Ln)
                dscr = wk7.tile([BL, B], F32, tag="dscr")
                dg = wk7.tile([BL, 1], F32, tag="dg")
                nc.vector.scalar_tensor_tensor(
                    dscr[:], sim_sb[:], 1.0, dmask_sb[:],
                    op0=ALU.mult, op1=ALU.mult, accum_out=dg[:])
                # v = 2*dg - (lg - nrmax) = 2*dg - lg + nrmax  [BL,1]
                v1 = wk7.tile([BL, 1], F32, tag="v1")
                nc.vector.scalar_tensor_tensor(
                    v1[:], dg[:], 2.0, lg[:],
                    op0=ALU.mult, op1=ALU.subtract)
                v2 = wk7.tile([BL, 1], F32, tag="v2")
                nc.vector.tensor_tensor(v2[:], v1[:], nrmax[:], op=ALU.add)
                # column partials: sum_i e^{sim[i, j]} (no shift)
                ecol = wk7.tile([BL, B], BF16, tag="ecol")
                nc.scalar.activation(ecol[:], sim_sb[:], ACTF.Exp)
                csum = ps7.tile([1, B], F32, tag="csum")
                nc.tensor.matmul(csum[:], lhsT=ones32b_sb[:],
                                 rhs=ecol[:], start=True, stop=True)
                ssum = ps7.tile([1, 1], F32, tag="ssum")
                nc.tensor.matmul(ssum[:], lhsT=ones32_sb[:], rhs=v2[:],
                                 start=True, stop=True)
                arv = wk7.tile([1, B + 1], F32, tag="arv")
                nc.scalar.copy(arv[0:1, 0:B], csum[0:1, :])
                nc.scalar.copy(arv[0:1, B:B + 1], ssum[0:1, :])
                nc.sync.dma_start(ar_in[:], arv[:])
                nc.gpsimd.collective_compute(
                    "AllReduce", ALU.add,
                    ins=[ar_in[:]], outs=[ar_out[:]],
                    replica_groups=[list(range(NCORES))])
                arr = wk7.tile([1, B + 1], F32, tag="arr")
                nc.sync.dma_start(arr[:], ar_out[:])
                lnc = wk7.tile([1, B], F32, tag="lnc")
                lnsum = wk7.tile([1, 1], F32, tag="lnsum")
                nc.scalar.activation(lnc[:], arr[0:1, 0:B], ACTF.Ln,
                                     accum_out=lnsum[:])
                fin = wk7.tile([1, 1], F32, tag="fin")
                nc.vector.tensor_tensor(fin[:], lnsum[:],
                                        arr[0:1, B:B + 1],
                                        op=ALU.subtract)
                osb = wk7.tile([1, 1], F32, tag="osb")
                nc.scalar.mul(osb[:], fin[:], 1.0 / (2 * B))
                nc.sync.dma_start(out[:], osb[0:1, :])

    return nc


def _host_prep(inputs):
    concept_feat = np.ascontiguousarray(np.asarray(inputs["concept_feat"],
                                                   dtype=np.float32))
    text_embeds = np.array(np.asarray(inputs["text_embeds"],
                                      dtype=np.float32), copy=True)
    text_mask = np.asarray(inputs["text_mask"]).astype(np.int32)
    Wc = np.ascontiguousarray(np.asarray(inputs["Wc"], dtype=np.float32))
    bc = np.asarray(inputs["bc"], dtype=np.float32)
    Ww = np.ascontiguousarray(np.asarray(inputs["Ww"], dtype=np.float32))
    bw = np.asarray(inputs["bw"], dtype=np.float32)
    temp = float(np.asarray(inputs["temp_cpt"]))

    # word mask (drop CLS + SEP), valid counts
    m = text_mask.copy()
    m[:, 0] = 0
    sep = (L - 1) - np.argmax(m[:, ::-1] > 0, axis=1)
    m[np.arange(B), sep] = 0
    nw = m.sum(axis=1).astype(np.float32)

    # sanitize invalid text rows with copies of position 1 (always valid)
    for j in range(B):
        inv = m[j] == 0
        text_embeds[j, inv] = text_embeds[j, 1]

    # Eq: partition p=(qo,i) -> i one-hot; eones adds 1/(Q*temp*FSC^2)
    eq = np.zeros((128, BL), dtype=np.float32)
    eones = np.zeros((128, BL), dtype=np.float32)
    for p in range(128):
        eq[p, p % BL] = 1.0
        eones[p, p % BL] = 1.0 / (Q * temp * FSC * FSC)

    # maskw[i, (j,l)] = m[j,l] / (nw_j * temp * FSC^2 * BHAT), bcast over i
    w_jl = (m.astype(np.float32) /
            (nw[:, None] * temp * FSC * FSC * BHAT)).reshape(1, JL)
    maskw = np.repeat(w_jl, BL, axis=0)

    ident = np.eye(128, dtype=np.float32)
    ones_row = np.ones((1, 128), dtype=np.float32)
    ones32 = np.ones((BL, 1), dtype=np.float32)
    brows = np.concatenate([bc, bw])[None, :]

    shared = {
        "wc": Wc.astype(ml_dtypes.bfloat16), "ww": Ww.astype(ml_dtypes.bfloat16),
        "brows": brows.astype(ml_dtypes.bfloat16),
        "ones_row": ones_row.astype(ml_dtypes.bfloat16),
        "ident_bf": ident.astype(ml_dtypes.bfloat16),
        "eqmat": eq.astype(ml_dtypes.bfloat16),
        "eones": eones.astype(ml_dtypes.bfloat16),
        "maskw": maskw.astype(ml_dtypes.bfloat16),
        "ones32": ones32,
    }
    in_maps = []
    for r in range(NCORES):
        im = dict(shared)
        im["concept_t"] = np.ascontiguousarray(
            concept_feat[r * BL:(r + 1) * BL].transpose(1, 0, 2)
            .reshape(IQ, VW).T).astype(ml_dtypes.bfloat16)
        im["text_t"] = np.ascontiguousarray(
            text_embeds[r * BL:(r + 1) * BL].reshape(JLL, TW).T
        ).astype(ml_dtypes.bfloat16)
        dmask_np = np.zeros((BL, B), dtype=np.float32)
        dmask_np[np.arange(BL), r * BL + np.arange(BL)] = 1.0
        im["dmask"] = dmask_np
        in_maps.append(im)
    return in_maps


def kernel(**inputs):
    in_maps = _host_prep(inputs)
    if "nc" not in _CACHE:
        _CACHE["nc"] = _build()
    res = run_bass_kernel_spmd(_CACHE["nc"], in_maps,
                               core_ids=list(range(NCORES)))
    return np.float32(res.results[0]["out"][0, 0])
